# revision 10
# baseline (speedup 1.0000x reference)
"""DCN (deep & cross network) inference kernel for 8 trn2 NeuronCores.

Strategy
--------
Data-parallel over the batch: each of the 8 cores processes 2048 of the
16384 rows.  The cross network is collapsed algebraically:

    xl_{i+1} = x0 * (xl_i . w_i) + b_i + xl_i   (x0 = x)
    =>  xl_3 = x * (1 + S) + (b0+b1+b2)

with S a per-row scalar computable from u_i = x . w_i plus constants
c_ij = b_i . w_j.  Only xl_3 . w_out[:1024] feeds the output, so the
whole cross network reduces to 4 per-row dot products u0..u3
(u3 = x . w_out[:1024]) and ~15 scalar ops per row; those dots are a
[16384,1024]x[1024,4] sgemm the host does in fp32 (precision matters
there - the u's multiply each other - and it is 6% of total flops).

The device runs the deep tower in feature-major layout (features on
partitions, rows on the free axis), with BatchNorm folded into the
following matmul's weights/bias:

    Z.T [64, N]  = w1.T @ x.T                     (the 2.1 GFLOP matmul)
    r   [64, N]  = relu(Z.T + b1)
    t2  [48, N]  = tanh(W2'.T @ r + b2')
    t3  [24, N]  = tanh(W3'.T @ t2 + b3')   -> returned per core

Matmuls run in float32r (fp32 rounded to 11 mantissa bits; 1 PE
cycle/column vs fp32's 4) with host-side round-to-nearest-even.  The
relu/tanh chain compresses the ~1e-4 rounding noise, so the final
output error stays at the few-1e-4 level.  x is transposed on the host
so the PE streams it without any on-chip transpose; a burst of dummy
warm-up matmuls during the first x-block's DMA gets the PE HAM clock
gate to 8/8 before real work arrives.  The host finishes with
hd = (a3*w_out_h) . t3, the cross-scalar recurrence and the sigmoid.
"""

import numpy as np

B, D = 16384, 1024
N_CORES = 8
ROWS = B // N_CORES          # rows per core
BS = 512                     # matmul free-dim block (moving-operand max)
NBLK = ROWS // BS
KT = D // 128                # number of 128-feature contraction tiles
NW = 64                      # tower width
N_WARMUP = 5                 # dummy matmuls to warm the PE clock gate
CH = 2                       # k-tiles per x DMA chunk (512 KB)
NCH = KT // CH
EPS = 1e-3

# const layout inside the fused weight tensor [128, CW]
_W2_OFF = KT * NW            # 512
_W3_OFF = _W2_OFF + 48       # 560
_B_OFF = _W3_OFF + 24        # 584: b1, b2', b3' columns
CW = _B_OFF + 3              # 587

_STATE: dict = {}


def _round_fp32r(a: np.ndarray) -> np.ndarray:
    """Round-to-nearest-even fp32 -> fp32r (low 12 mantissa bits zero)."""
    u = np.ascontiguousarray(a, np.float32).view(np.uint32).copy()
    u += 0x7FF + ((u >> 12) & 1)
    u &= np.uint32(0xFFFFF000)
    return u.view(np.float32)


def _build_bass():
    import concourse.bacc as bacc
    import concourse.bass as bass
    import concourse.mybir as mybir
    import concourse.tile as tile

    f32 = mybir.dt.float32
    f32r = mybir.dt.float32r
    AFT = mybir.ActivationFunctionType

    nc = bacc.Bacc("TRN2", target_bir_lowering=False, debug=False)

    xt = nc.dram_tensor("xt", [D, ROWS], f32r, kind="ExternalInput")
    wts = nc.dram_tensor("wts", [128, CW], f32r, kind="ExternalInput")
    out3 = nc.dram_tensor("out3", [24, ROWS], f32, kind="ExternalOutput")

    with tile.TileContext(nc) as tc:
        with (
            tc.tile_pool(name="const", bufs=1) as cpool,
            tc.tile_pool(name="xin", bufs=10) as xpool,
            tc.tile_pool(name="act", bufs=3) as apool,
            tc.tile_pool(name="pz", bufs=2, space=bass.MemorySpace.PSUM) as pz,
            tc.tile_pool(name="p2", bufs=2, space=bass.MemorySpace.PSUM) as p2,
            tc.tile_pool(name="p3", bufs=2, space=bass.MemorySpace.PSUM) as p3,
            tc.tile_pool(name="pw", bufs=1, space=bass.MemorySpace.PSUM) as pw,
        ):
            w_t = cpool.tile([128, CW], f32r)
            nc.sync.dma_start(w_t[:], wts[:])

            W2 = w_t[0:64, _W2_OFF:_W2_OFF + 48]
            W3 = w_t[0:48, _W3_OFF:_W3_OFF + 24]
            B1 = w_t[0:64, _B_OFF + 0:_B_OFF + 1].bitcast(f32)
            B2 = w_t[0:48, _B_OFF + 1:_B_OFF + 2].bitcast(f32)
            B3 = w_t[0:24, _B_OFF + 2:_B_OFF + 3].bitcast(f32)

            def wk(k):
                return w_t[:, k * NW:(k + 1) * NW]

            # PE warm-up: dummy matmuls on the (already loaded) weights so
            # the HAM clock gate reaches 8/8 while the first x chunks DMA.
            wm = pw.tile([NW, BS], f32)
            for _ in range(N_WARMUP):
                nc.tensor.matmul(wm[:], wk(0), w_t[:, 0:BS], start=True, stop=True)

            xt_v = xt.ap().rearrange("(k p) n -> p k n", p=128)  # [128, KT, ROWS]

            for b in range(NBLK):
                cols = slice(b * BS, (b + 1) * BS)
                # stream the block in CH-k-tile chunks so the PE starts as
                # soon as the first chunk lands and DMA never stalls
                chunks = []
                for j in range(NCH):
                    xc = xpool.tile([128, CH, BS], f32r, tag="xc")
                    nc.sync.dma_start(
                        xc[:], xt_v[:, j * CH:(j + 1) * CH, cols])
                    chunks.append(xc)

                zt = pz.tile([NW, BS], f32)
                for k in range(KT):
                    nc.tensor.matmul(
                        zt[:], wk(k), chunks[k // CH][:, k % CH, :],
                        start=(k == 0), stop=(k == KT - 1),
                    )

                r = apool.tile([64, BS], f32r, tag="r")
                nc.scalar.activation(r[:], zt[:], AFT.Relu, bias=B1)

                z2 = p2.tile([48, BS], f32)
                nc.tensor.matmul(z2[:], W2, r[:], start=True, stop=True)
                t2 = apool.tile([48, BS], f32r, tag="t2")
                nc.scalar.activation(t2[:], z2[:], AFT.Tanh, bias=B2)

                z3 = p3.tile([24, BS], f32)
                nc.tensor.matmul(z3[:], W3, t2[:], start=True, stop=True)
                t3 = apool.tile([24, BS], f32, tag="t3")
                nc.scalar.activation(t3[:], z3[:], AFT.Tanh, bias=B3)

                nc.sync.dma_start(out3[:, cols], t3[:])

    nc.compile()
    return nc


def _get_nc():
    if "nc" not in _STATE:
        _STATE["nc"] = _build_bass()
    return _STATE["nc"]


def _prep(inputs):
    """Host-side folding of the tiny weights + the fp32 u-sgemm."""
    f32 = np.float32
    x = np.asarray(inputs["x"], f32)
    cw = np.asarray(inputs["cross_w"], f32)
    cb = np.asarray(inputs["cross_b"], f32)
    w1 = np.asarray(inputs["w1"], f32)
    b1 = np.asarray(inputs["b1"], f32)
    w2 = np.asarray(inputs["w2"], f32)
    b2 = np.asarray(inputs["b2"], f32)
    w3 = np.asarray(inputs["w3"], f32)
    b3 = np.asarray(inputs["b3"], f32)
    w_out = np.asarray(inputs["w_out"], f32)
    b_out = np.asarray(inputs["b_out"], f32)

    def bn_fold(g, be, m, v):
        a = (np.asarray(g, np.float64) / np.sqrt(np.asarray(v, np.float64) + EPS))
        c = np.asarray(be, np.float64) - a * np.asarray(m, np.float64)
        return a, c

    a1, c1 = bn_fold(inputs["gamma1"], inputs["beta1"], inputs["mean1"], inputs["var1"])
    a2, c2 = bn_fold(inputs["gamma2"], inputs["beta2"], inputs["mean2"], inputs["var2"])
    a3, c3 = bn_fold(inputs["gamma3"], inputs["beta3"], inputs["mean3"], inputs["var3"])

    w_out_x = w_out[:D, 0]
    w_out_h = w_out[D:, 0]

    W2p = (a1[:, None] * w2).astype(f32)                  # [64, 48]
    b2p = (c1 @ w2 + b2).astype(f32)                      # [48]
    W3p = (a2[:, None] * w3).astype(f32)                  # [48, 24]
    b3p = (c2 @ w3 + b3).astype(f32)                      # [24]
    wh = (a3 * w_out_h).astype(f32)                       # [24]
    ch = float(c3 @ w_out_h)

    c01 = float(cb[0] @ cw[1])
    c02 = float(cb[0] @ cw[2])
    c12 = float(cb[1] @ cw[2])
    c3s = float(cb.sum(axis=0) @ w_out_x)

    # the 4 cross dot products, exact fp32 on host (6% of total flops)
    Wc = np.stack([cw[0], cw[1], cw[2], w_out_x], axis=1).astype(f32)   # [D, 4]
    U = x @ Wc                                                          # [B, 4]

    # fused device-side const tensor
    wts = np.zeros((128, CW), f32)
    w1p = _round_fp32r(w1)                                # [D, 64] k-major pack
    wts[:, :KT * NW] = w1p.reshape(KT, 128, NW).transpose(1, 0, 2).reshape(128, -1)
    wts[0:64, _W2_OFF:_W2_OFF + 48] = _round_fp32r(W2p)
    wts[0:48, _W3_OFF:_W3_OFF + 24] = _round_fp32r(W3p)
    wts[0:64, _B_OFF + 0] = b1
    wts[0:48, _B_OFF + 1] = b2p
    wts[0:24, _B_OFF + 2] = b3p

    consts = dict(c01=c01, c02=c02, c12=c12, c3s=c3s, ch=ch,
                  b_out=float(b_out[0]), wh=wh, U=U)
    return x, wts, consts


def _combine(t3_all, consts):
    """t3_all: [24, B] device tower output -> final sigmoid output [B, 1]."""
    U = consts["U"].astype(np.float64)
    u0, u1, u2, u3 = U[:, 0], U[:, 1], U[:, 2], U[:, 3]
    hd = consts["wh"].astype(np.float64) @ t3_all.astype(np.float64)     # [B]
    oneS = ((1.0 + u0) * (1.0 + u1) + consts["c01"]) * (1.0 + u2) \
        + consts["c02"] + consts["c12"]
    lin = oneS * u3 + consts["c3s"] + hd + consts["ch"] + consts["b_out"]
    y = 1.0 / (1.0 + np.exp(-lin))
    return y.reshape(-1, 1).astype(np.float32)


def _run(inputs, trace=False, **spmd_kwargs):
    from concourse.bass_utils import run_bass_kernel_spmd

    x, wts, consts = _prep(inputs)
    nc = _get_nc()

    in_maps = []
    for c in range(N_CORES):
        xt_c = _round_fp32r(x[c * ROWS:(c + 1) * ROWS, :].T)
        in_maps.append({"xt": xt_c, "wts": wts})

    res = run_bass_kernel_spmd(
        nc, in_maps, core_ids=list(range(N_CORES)), trace=trace, **spmd_kwargs
    )
    t3_all = np.concatenate([r["out3"] for r in res.results], axis=1)  # [24, B]
    return _combine(t3_all, consts), res


def kernel(**inputs) -> np.ndarray:
    y, _ = _run(inputs, trace=False)
    return y


# revision 13
# speedup vs baseline: 1.1076x; 1.1076x over previous
"""DCN (deep & cross network) inference kernel for 8 trn2 NeuronCores.

Strategy
--------
Data-parallel over the batch: each of the 8 cores processes 2048 of the
16384 rows.  The cross network is collapsed algebraically:

    xl_{i+1} = x0 * (xl_i . w_i) + b_i + xl_i   (x0 = x)
    =>  xl_3 = x * (1 + S) + (b0+b1+b2)

with S a per-row scalar computable from u_i = x . w_i plus constants
c_ij = b_i . w_j.  Only xl_3 . w_out[:1024] feeds the output, so the
whole cross network reduces to 4 per-row dot products u0..u3
(u3 = x . w_out[:1024]) and ~15 scalar ops per row; those dots are a
[16384,1024]x[1024,4] sgemm the host does in fp32 (precision matters
there - the u's multiply each other - and it is 6% of total flops).

The device runs the deep tower in feature-major layout (features on
partitions, rows on the free axis), with BatchNorm folded into the
following matmul's weights/bias:

    Z.T [64, N]  = w1.T @ x.T                     (the 2.1 GFLOP matmul)
    r   [64, N]  = relu(Z.T + b1)
    t2  [48, N]  = tanh(W2'.T @ r + b2')
    t3  [24, N]  = tanh(W3'.T @ t2 + b3')   -> returned per core

Matmuls run in float32r (fp32 rounded to 11 mantissa bits; 1 PE
cycle/column vs fp32's 4) with host-side round-to-nearest-even.  The
relu/tanh chain compresses the ~1e-4 rounding noise, so the final
output error stays at the few-1e-4 level.  x is transposed on the host
so the PE streams it without any on-chip transpose; a burst of dummy
warm-up matmuls during the first x-block's DMA gets the PE HAM clock
gate to 8/8 before real work arrives.  The host finishes with
hd = (a3*w_out_h) . t3, the cross-scalar recurrence and the sigmoid.
"""

import numpy as np

B, D = 16384, 1024
N_CORES = 8
ROWS = B // N_CORES          # rows per core
BS = 512                     # matmul free-dim block (moving-operand max)
NBLK = ROWS // BS
KT = D // 128                # number of 128-feature contraction tiles
NW = 64                      # tower width
N_WARMUP = 8                 # dummy matmuls to warm the PE clock gate
CH = 2                       # k-tiles per x DMA chunk (512 KB)
NCH = KT // CH
EPS = 1e-3

# const layout inside the fused weight tensor [128, CW]
_W2_OFF = KT * NW            # 512
_W3_OFF = _W2_OFF + 48       # 560
_B_OFF = _W3_OFF + 24        # 584: b1, b2', b3' columns
CW = _B_OFF + 3              # 587

_STATE: dict = {}


def _round_fp32r(a: np.ndarray) -> np.ndarray:
    """Round-to-nearest-even fp32 -> fp32r (low 12 mantissa bits zero)."""
    u = np.ascontiguousarray(a, np.float32).view(np.uint32).copy()
    u += 0x7FF + ((u >> 12) & 1)
    u &= np.uint32(0xFFFFF000)
    return u.view(np.float32)


def _build_bass():
    import concourse.bacc as bacc
    import concourse.bass as bass
    import concourse.mybir as mybir
    import concourse.tile as tile

    f32 = mybir.dt.float32
    f32r = mybir.dt.float32r
    AFT = mybir.ActivationFunctionType

    nc = bacc.Bacc("TRN2", target_bir_lowering=False, debug=False)

    xt = nc.dram_tensor("xt", [D, ROWS], f32r, kind="ExternalInput")
    wts = nc.dram_tensor("wts", [128, CW], f32r, kind="ExternalInput")
    out3 = nc.dram_tensor("out3", [24, ROWS], f32, kind="ExternalOutput")

    with tile.TileContext(nc) as tc:
        with (
            tc.tile_pool(name="const", bufs=1) as cpool,
            tc.tile_pool(name="xin", bufs=10) as xpool,
            tc.tile_pool(name="act", bufs=3) as apool,
            tc.tile_pool(name="pz", bufs=2, space=bass.MemorySpace.PSUM) as pz,
            tc.tile_pool(name="p2", bufs=2, space=bass.MemorySpace.PSUM) as p2,
            tc.tile_pool(name="p3", bufs=2, space=bass.MemorySpace.PSUM) as p3,
            tc.tile_pool(name="pw", bufs=1, space=bass.MemorySpace.PSUM) as pw,
        ):
            w_t = cpool.tile([128, CW], f32r)
            nc.sync.dma_start(w_t[:], wts[:])

            W2 = w_t[0:64, _W2_OFF:_W2_OFF + 48]
            W3 = w_t[0:48, _W3_OFF:_W3_OFF + 24]
            B1 = w_t[0:64, _B_OFF + 0:_B_OFF + 1].bitcast(f32)
            B2 = w_t[0:48, _B_OFF + 1:_B_OFF + 2].bitcast(f32)
            B3 = w_t[0:24, _B_OFF + 2:_B_OFF + 3].bitcast(f32)

            def wk(k):
                return w_t[:, k * NW:(k + 1) * NW]

            # PE warm-up: dummy matmuls on the (already loaded) weights so
            # the HAM clock gate reaches 8/8 while the first x chunks DMA.
            wm = pw.tile([NW, BS], f32)
            for _ in range(N_WARMUP):
                nc.tensor.matmul(wm[:], wk(0), w_t[:, 0:BS], start=True, stop=True)

            xt_v = xt.ap().rearrange("(k p) n -> p k n", p=128)  # [128, KT, ROWS]

            for b in range(NBLK):
                cols = slice(b * BS, (b + 1) * BS)
                # stream the block in CH-k-tile chunks so the PE starts as
                # soon as the first chunk lands and DMA never stalls
                chunks = []
                for j in range(NCH):
                    xc = xpool.tile([128, CH, BS], f32r, tag="xc")
                    nc.sync.dma_start(
                        xc[:], xt_v[:, j * CH:(j + 1) * CH, cols])
                    chunks.append(xc)

                zt = pz.tile([NW, BS], f32)
                for k in range(KT):
                    nc.tensor.matmul(
                        zt[:], wk(k), chunks[k // CH][:, k % CH, :],
                        start=(k == 0), stop=(k == KT - 1),
                    )

                r = apool.tile([64, BS], f32r, tag="r")
                nc.vector.tensor_scalar(
                    r[:], zt[:], B1, 0.0,
                    mybir.AluOpType.add, mybir.AluOpType.max,
                )

                z2 = p2.tile([48, BS], f32)
                nc.tensor.matmul(z2[:], W2, r[:], start=True, stop=True)
                t2 = apool.tile([48, BS], f32r, tag="t2")
                nc.scalar.activation(t2[:], z2[:], AFT.Tanh, bias=B2)

                z3 = p3.tile([24, BS], f32)
                nc.tensor.matmul(z3[:], W3, t2[:], start=True, stop=True)
                t3 = apool.tile([24, BS], f32, tag="t3")
                nc.scalar.activation(t3[:], z3[:], AFT.Tanh, bias=B3)

                nc.scalar.dma_start(out3[:, cols], t3[:])

    nc.compile()
    return nc


def _get_nc():
    if "nc" not in _STATE:
        _STATE["nc"] = _build_bass()
    return _STATE["nc"]


def _prep(inputs):
    """Host-side folding of the tiny weights + the fp32 u-sgemm."""
    f32 = np.float32
    x = np.asarray(inputs["x"], f32)
    cw = np.asarray(inputs["cross_w"], f32)
    cb = np.asarray(inputs["cross_b"], f32)
    w1 = np.asarray(inputs["w1"], f32)
    b1 = np.asarray(inputs["b1"], f32)
    w2 = np.asarray(inputs["w2"], f32)
    b2 = np.asarray(inputs["b2"], f32)
    w3 = np.asarray(inputs["w3"], f32)
    b3 = np.asarray(inputs["b3"], f32)
    w_out = np.asarray(inputs["w_out"], f32)
    b_out = np.asarray(inputs["b_out"], f32)

    def bn_fold(g, be, m, v):
        a = (np.asarray(g, np.float64) / np.sqrt(np.asarray(v, np.float64) + EPS))
        c = np.asarray(be, np.float64) - a * np.asarray(m, np.float64)
        return a, c

    a1, c1 = bn_fold(inputs["gamma1"], inputs["beta1"], inputs["mean1"], inputs["var1"])
    a2, c2 = bn_fold(inputs["gamma2"], inputs["beta2"], inputs["mean2"], inputs["var2"])
    a3, c3 = bn_fold(inputs["gamma3"], inputs["beta3"], inputs["mean3"], inputs["var3"])

    w_out_x = w_out[:D, 0]
    w_out_h = w_out[D:, 0]

    W2p = (a1[:, None] * w2).astype(f32)                  # [64, 48]
    b2p = (c1 @ w2 + b2).astype(f32)                      # [48]
    W3p = (a2[:, None] * w3).astype(f32)                  # [48, 24]
    b3p = (c2 @ w3 + b3).astype(f32)                      # [24]
    wh = (a3 * w_out_h).astype(f32)                       # [24]
    ch = float(c3 @ w_out_h)

    c01 = float(cb[0] @ cw[1])
    c02 = float(cb[0] @ cw[2])
    c12 = float(cb[1] @ cw[2])
    c3s = float(cb.sum(axis=0) @ w_out_x)

    # the 4 cross dot products, exact fp32 on host (6% of total flops)
    Wc = np.stack([cw[0], cw[1], cw[2], w_out_x], axis=1).astype(f32)   # [D, 4]
    U = x @ Wc                                                          # [B, 4]

    # fused device-side const tensor
    wts = np.zeros((128, CW), f32)
    w1p = _round_fp32r(w1)                                # [D, 64] k-major pack
    wts[:, :KT * NW] = w1p.reshape(KT, 128, NW).transpose(1, 0, 2).reshape(128, -1)
    wts[0:64, _W2_OFF:_W2_OFF + 48] = _round_fp32r(W2p)
    wts[0:48, _W3_OFF:_W3_OFF + 24] = _round_fp32r(W3p)
    wts[0:64, _B_OFF + 0] = b1
    wts[0:48, _B_OFF + 1] = b2p
    wts[0:24, _B_OFF + 2] = b3p

    consts = dict(c01=c01, c02=c02, c12=c12, c3s=c3s, ch=ch,
                  b_out=float(b_out[0]), wh=wh, U=U)
    return x, wts, consts


def _combine(t3_all, consts):
    """t3_all: [24, B] device tower output -> final sigmoid output [B, 1]."""
    U = consts["U"].astype(np.float64)
    u0, u1, u2, u3 = U[:, 0], U[:, 1], U[:, 2], U[:, 3]
    hd = consts["wh"].astype(np.float64) @ t3_all.astype(np.float64)     # [B]
    oneS = ((1.0 + u0) * (1.0 + u1) + consts["c01"]) * (1.0 + u2) \
        + consts["c02"] + consts["c12"]
    lin = oneS * u3 + consts["c3s"] + hd + consts["ch"] + consts["b_out"]
    y = 1.0 / (1.0 + np.exp(-lin))
    return y.reshape(-1, 1).astype(np.float32)


def _run(inputs, trace=False, **spmd_kwargs):
    from concourse.bass_utils import run_bass_kernel_spmd

    x, wts, consts = _prep(inputs)
    nc = _get_nc()

    in_maps = []
    for c in range(N_CORES):
        xt_c = _round_fp32r(x[c * ROWS:(c + 1) * ROWS, :].T)
        in_maps.append({"xt": xt_c, "wts": wts})

    res = run_bass_kernel_spmd(
        nc, in_maps, core_ids=list(range(N_CORES)), trace=trace, **spmd_kwargs
    )
    t3_all = np.concatenate([r["out3"] for r in res.results], axis=1)  # [24, B]
    return _combine(t3_all, consts), res


def kernel(**inputs) -> np.ndarray:
    y, _ = _run(inputs, trace=False)
    return y


# revision 15
# speedup vs baseline: 1.1582x; 1.0456x over previous
"""DCN (deep & cross network) inference kernel for 8 trn2 NeuronCores.

Strategy
--------
Data-parallel over the batch: each of the 8 cores processes 2048 of the
16384 rows.  The cross network is collapsed algebraically:

    xl_{i+1} = x0 * (xl_i . w_i) + b_i + xl_i   (x0 = x)
    =>  xl_3 = x * (1 + S) + (b0+b1+b2)

with S a per-row scalar computable from u_i = x . w_i plus constants
c_ij = b_i . w_j.  Only xl_3 . w_out[:1024] feeds the output, so the
whole cross network reduces to 4 per-row dot products u0..u3
(u3 = x . w_out[:1024]) and ~15 scalar ops per row; those dots are a
[16384,1024]x[1024,4] sgemm the host does in fp32 (precision matters
there - the u's multiply each other - and it is 6% of total flops).

The device runs the deep tower in feature-major layout (features on
partitions, rows on the free axis), with BatchNorm folded into the
following matmul's weights/bias:

    Z.T [64, N]  = w1.T @ x.T                     (the 2.1 GFLOP matmul)
    r   [64, N]  = relu(Z.T + b1)
    t2  [48, N]  = tanh(W2'.T @ r + b2')
    t3  [24, N]  = tanh(W3'.T @ t2 + b3')   -> returned per core

Matmuls run in float32r (fp32 rounded to 11 mantissa bits; 1 PE
cycle/column vs fp32's 4) with host-side round-to-nearest-even.  The
relu/tanh chain compresses the ~1e-4 rounding noise, so the final
output error stays at the few-1e-4 level.  x is transposed on the host
so the PE streams it without any on-chip transpose; a burst of dummy
warm-up matmuls during the first x-block's DMA gets the PE HAM clock
gate to 8/8 before real work arrives.  The host finishes with
hd = (a3*w_out_h) . t3, the cross-scalar recurrence and the sigmoid.
"""

import numpy as np

B, D = 16384, 1024
N_CORES = 8
ROWS = B // N_CORES          # rows per core
BS = 512                     # matmul free-dim block (moving-operand max)
NBLK = ROWS // BS
KT = D // 128                # number of 128-feature contraction tiles
NW = 64                      # tower width
N_WARMUP = 6                 # dummy matmuls to warm the PE clock gate
CH = 2                       # k-tiles per x DMA chunk (512 KB)
NCH = KT // CH
EPS = 1e-3

# const layout inside the fused weight tensor [128, CW]
_W2_OFF = KT * NW            # 512
_W3_OFF = _W2_OFF + 48       # 560
_B_OFF = _W3_OFF + 24        # 584: b1, b2', b3' columns
CW = _B_OFF + 3              # 587

_STATE: dict = {}


def _round_fp32r(a: np.ndarray) -> np.ndarray:
    """Round-to-nearest-even fp32 -> fp32r (low 12 mantissa bits zero)."""
    u = np.ascontiguousarray(a, np.float32).view(np.uint32).copy()
    u += 0x7FF + ((u >> 12) & 1)
    u &= np.uint32(0xFFFFF000)
    return u.view(np.float32)


def _build_bass():
    import concourse.bacc as bacc
    import concourse.bass as bass
    import concourse.mybir as mybir
    import concourse.tile as tile

    f32 = mybir.dt.float32
    f32r = mybir.dt.float32r
    AFT = mybir.ActivationFunctionType

    nc = bacc.Bacc("TRN2", target_bir_lowering=False, debug=False)

    xt = nc.dram_tensor("xt", [D, ROWS], f32r, kind="ExternalInput")
    wts = nc.dram_tensor("wts", [128, CW], f32r, kind="ExternalInput")
    out3 = nc.dram_tensor("out3", [24, ROWS], f32, kind="ExternalOutput")

    with tile.TileContext(nc) as tc:
        with (
            tc.tile_pool(name="const", bufs=1) as cpool,
            tc.tile_pool(name="xin", bufs=16) as xpool,
            tc.tile_pool(name="act", bufs=4) as apool,
            tc.tile_pool(name="pz", bufs=2, space=bass.MemorySpace.PSUM) as pz,
            tc.tile_pool(name="p2", bufs=2, space=bass.MemorySpace.PSUM) as p2,
            tc.tile_pool(name="p3", bufs=2, space=bass.MemorySpace.PSUM) as p3,
            tc.tile_pool(name="pw", bufs=1, space=bass.MemorySpace.PSUM) as pw,
        ):
            w_t = cpool.tile([128, CW], f32r)
            nc.sync.dma_start(w_t[:], wts[:])

            W2 = w_t[0:64, _W2_OFF:_W2_OFF + 48]
            W3 = w_t[0:48, _W3_OFF:_W3_OFF + 24]
            B1 = w_t[0:64, _B_OFF + 0:_B_OFF + 1].bitcast(f32)
            B2 = w_t[0:48, _B_OFF + 1:_B_OFF + 2].bitcast(f32)
            B3 = w_t[0:24, _B_OFF + 2:_B_OFF + 3].bitcast(f32)

            def wk(k):
                return w_t[:, k * NW:(k + 1) * NW]

            # PE warm-up: dummy matmuls on the (already loaded) weights so
            # the HAM clock gate reaches 8/8 while the first x chunks DMA.
            wm = pw.tile([NW, BS], f32)
            for _ in range(N_WARMUP):
                nc.tensor.matmul(wm[:], wk(0), w_t[:, 0:BS], start=True, stop=True)

            xt_v = xt.ap().rearrange("(k p) n -> p k n", p=128)  # [128, KT, ROWS]

            rs: dict = {}
            t2s: dict = {}

            def tower2(i):
                # mm2 + tanh for block i (relu(i) finished a block ago, so
                # the PE never stalls on the activation chain)
                z2 = p2.tile([48, BS], f32)
                nc.tensor.matmul(z2[:], W2, rs[i][:], start=True, stop=True)
                t2 = apool.tile([48, BS], f32r, tag="t2")
                nc.scalar.activation(t2[:], z2[:], AFT.Tanh, bias=B2)
                t2s[i] = t2

            def tower3(i):
                z3 = p3.tile([24, BS], f32)
                nc.tensor.matmul(z3[:], W3, t2s[i][:], start=True, stop=True)
                t3 = apool.tile([24, BS], f32, tag="t3")
                nc.scalar.activation(t3[:], z3[:], AFT.Tanh, bias=B3)
                nc.scalar.dma_start(out3[:, i * BS:(i + 1) * BS], t3[:])

            for b in range(NBLK):
                cols = slice(b * BS, (b + 1) * BS)
                # stream the block in CH-k-tile chunks so the PE starts as
                # soon as the first chunk lands and DMA never stalls
                chunks = []
                for j in range(NCH):
                    xc = xpool.tile([128, CH, BS], f32r, tag="xc")
                    nc.sync.dma_start(
                        xc[:], xt_v[:, j * CH:(j + 1) * CH, cols])
                    chunks.append(xc)

                zt = pz.tile([NW, BS], f32)
                for k in range(KT):
                    nc.tensor.matmul(
                        zt[:], wk(k), chunks[k // CH][:, k % CH, :],
                        start=(k == 0), stop=(k == KT - 1),
                    )

                r = apool.tile([64, BS], f32r, tag="r")
                nc.vector.tensor_scalar(
                    r[:], zt[:], B1, 0.0,
                    mybir.AluOpType.add, mybir.AluOpType.max,
                )
                rs[b] = r

                if b >= 1:
                    tower2(b - 1)
                if b >= 2:
                    tower3(b - 2)

            tower2(NBLK - 1)
            tower3(NBLK - 2)
            tower3(NBLK - 1)

    nc.compile()
    return nc


def _get_nc():
    if "nc" not in _STATE:
        _STATE["nc"] = _build_bass()
    return _STATE["nc"]


def _prep(inputs):
    """Host-side folding of the tiny weights + the fp32 u-sgemm."""
    f32 = np.float32
    x = np.asarray(inputs["x"], f32)
    cw = np.asarray(inputs["cross_w"], f32)
    cb = np.asarray(inputs["cross_b"], f32)
    w1 = np.asarray(inputs["w1"], f32)
    b1 = np.asarray(inputs["b1"], f32)
    w2 = np.asarray(inputs["w2"], f32)
    b2 = np.asarray(inputs["b2"], f32)
    w3 = np.asarray(inputs["w3"], f32)
    b3 = np.asarray(inputs["b3"], f32)
    w_out = np.asarray(inputs["w_out"], f32)
    b_out = np.asarray(inputs["b_out"], f32)

    def bn_fold(g, be, m, v):
        a = (np.asarray(g, np.float64) / np.sqrt(np.asarray(v, np.float64) + EPS))
        c = np.asarray(be, np.float64) - a * np.asarray(m, np.float64)
        return a, c

    a1, c1 = bn_fold(inputs["gamma1"], inputs["beta1"], inputs["mean1"], inputs["var1"])
    a2, c2 = bn_fold(inputs["gamma2"], inputs["beta2"], inputs["mean2"], inputs["var2"])
    a3, c3 = bn_fold(inputs["gamma3"], inputs["beta3"], inputs["mean3"], inputs["var3"])

    w_out_x = w_out[:D, 0]
    w_out_h = w_out[D:, 0]

    W2p = (a1[:, None] * w2).astype(f32)                  # [64, 48]
    b2p = (c1 @ w2 + b2).astype(f32)                      # [48]
    W3p = (a2[:, None] * w3).astype(f32)                  # [48, 24]
    b3p = (c2 @ w3 + b3).astype(f32)                      # [24]
    wh = (a3 * w_out_h).astype(f32)                       # [24]
    ch = float(c3 @ w_out_h)

    c01 = float(cb[0] @ cw[1])
    c02 = float(cb[0] @ cw[2])
    c12 = float(cb[1] @ cw[2])
    c3s = float(cb.sum(axis=0) @ w_out_x)

    # the 4 cross dot products, exact fp32 on host (6% of total flops)
    Wc = np.stack([cw[0], cw[1], cw[2], w_out_x], axis=1).astype(f32)   # [D, 4]
    U = x @ Wc                                                          # [B, 4]

    # fused device-side const tensor
    wts = np.zeros((128, CW), f32)
    w1p = _round_fp32r(w1)                                # [D, 64] k-major pack
    wts[:, :KT * NW] = w1p.reshape(KT, 128, NW).transpose(1, 0, 2).reshape(128, -1)
    wts[0:64, _W2_OFF:_W2_OFF + 48] = _round_fp32r(W2p)
    wts[0:48, _W3_OFF:_W3_OFF + 24] = _round_fp32r(W3p)
    wts[0:64, _B_OFF + 0] = b1
    wts[0:48, _B_OFF + 1] = b2p
    wts[0:24, _B_OFF + 2] = b3p

    consts = dict(c01=c01, c02=c02, c12=c12, c3s=c3s, ch=ch,
                  b_out=float(b_out[0]), wh=wh, U=U)
    return x, wts, consts


def _combine(t3_all, consts):
    """t3_all: [24, B] device tower output -> final sigmoid output [B, 1]."""
    U = consts["U"].astype(np.float64)
    u0, u1, u2, u3 = U[:, 0], U[:, 1], U[:, 2], U[:, 3]
    hd = consts["wh"].astype(np.float64) @ t3_all.astype(np.float64)     # [B]
    oneS = ((1.0 + u0) * (1.0 + u1) + consts["c01"]) * (1.0 + u2) \
        + consts["c02"] + consts["c12"]
    lin = oneS * u3 + consts["c3s"] + hd + consts["ch"] + consts["b_out"]
    y = 1.0 / (1.0 + np.exp(-lin))
    return y.reshape(-1, 1).astype(np.float32)


def _run(inputs, trace=False, **spmd_kwargs):
    from concourse.bass_utils import run_bass_kernel_spmd

    x, wts, consts = _prep(inputs)
    nc = _get_nc()

    in_maps = []
    for c in range(N_CORES):
        xt_c = _round_fp32r(x[c * ROWS:(c + 1) * ROWS, :].T)
        in_maps.append({"xt": xt_c, "wts": wts})

    res = run_bass_kernel_spmd(
        nc, in_maps, core_ids=list(range(N_CORES)), trace=trace, **spmd_kwargs
    )
    t3_all = np.concatenate([r["out3"] for r in res.results], axis=1)  # [24, B]
    return _combine(t3_all, consts), res


def kernel(**inputs) -> np.ndarray:
    y, _ = _run(inputs, trace=False)
    return y


# revision 17
# speedup vs baseline: 1.4915x; 1.2878x over previous
"""DCN (deep & cross network) inference kernel for 8 trn2 NeuronCores.

Strategy
--------
Data-parallel over the batch: each of the 8 cores processes 2048 of the
16384 rows.  The cross network is collapsed algebraically:

    xl_{i+1} = x0 * (xl_i . w_i) + b_i + xl_i   (x0 = x)
    =>  xl_3 = x * (1 + S) + (b0+b1+b2)

with S a per-row scalar computable from u_i = x . w_i plus constants
c_ij = b_i . w_j.  Only xl_3 . w_out[:1024] feeds the output, so the
whole cross network reduces to 4 per-row dot products u0..u3
(u3 = x . w_out[:1024]) and ~15 scalar ops per row; those dots are a
[16384,1024]x[1024,4] sgemm the host does in fp32 (precision matters
there - the u's multiply each other - and it is 6% of total flops).

The device runs the deep tower in feature-major layout (features on
partitions, rows on the free axis), with BatchNorm folded into the
following matmul's weights/bias:

    Z.T [64, N]  = w1.T @ x.T                     (the 2.1 GFLOP matmul)
    r   [64, N]  = relu(Z.T + b1)
    t2  [48, N]  = tanh(W2'.T @ r + b2')
    t3  [24, N]  = tanh(W3'.T @ t2 + b3')   -> returned per core

Matmuls run in float32r (fp32 rounded to 11 mantissa bits; 1 PE
cycle/column vs fp32's 4) with host-side round-to-nearest-even.  The
relu/tanh chain compresses the ~1e-4 rounding noise, so the final
output error stays at the few-1e-4 level.  x is transposed on the host
so the PE streams it without any on-chip transpose; a burst of dummy
warm-up matmuls during the first x-block's DMA gets the PE HAM clock
gate to 8/8 before real work arrives.  The host finishes with
hd = (a3*w_out_h) . t3, the cross-scalar recurrence and the sigmoid.
"""

import numpy as np

B, D = 16384, 1024
N_CORES = 8
ROWS = B // N_CORES          # rows per core
BS = 512                     # matmul free-dim block (moving-operand max)
NBLK = ROWS // BS
KT = D // 128                # number of 128-feature contraction tiles
NW = 64                      # tower width
N_WARMUP = 6                 # dummy matmuls to warm the PE clock gate
CH = 4                       # k-tiles per x DMA chunk (512 KB)
NCH = KT // CH
EPS = 1e-3

# const layout inside the fused weight tensor [128, CW]
_W2_OFF = KT * NW            # 512
_W3_OFF = _W2_OFF + 48       # 560
_B_OFF = _W3_OFF + 24        # 584: b1, b2', b3' as f32 bit-pairs
CW = _B_OFF + 6              # 590

_STATE: dict = {}


def _round_fp32r(a: np.ndarray) -> np.ndarray:
    """Round-to-nearest-even fp32 -> fp32r (low 12 mantissa bits zero)."""
    u = np.ascontiguousarray(a, np.float32).view(np.uint32).copy()
    u += 0x7FF + ((u >> 12) & 1)
    u &= np.uint32(0xFFFFF000)
    return u.view(np.float32)


def _build_bass():
    import concourse.bacc as bacc
    import concourse.bass as bass
    import concourse.mybir as mybir
    import concourse.tile as tile

    f32 = mybir.dt.float32
    f32r = mybir.dt.float32r
    f16 = mybir.dt.float16
    AFT = mybir.ActivationFunctionType

    nc = bacc.Bacc("TRN2", target_bir_lowering=False, debug=False)

    xt = nc.dram_tensor("xt", [D, ROWS], f16, kind="ExternalInput")
    wts = nc.dram_tensor("wts", [128, CW], f16, kind="ExternalInput")
    out3 = nc.dram_tensor("out3", [24, ROWS], f32, kind="ExternalOutput")

    with tile.TileContext(nc) as tc:
        with (
            tc.tile_pool(name="const", bufs=1) as cpool,
            tc.tile_pool(name="xin", bufs=16) as xpool,
            tc.tile_pool(name="act", bufs=4) as apool,
            tc.tile_pool(name="pz", bufs=2, space=bass.MemorySpace.PSUM) as pz,
            tc.tile_pool(name="p2", bufs=2, space=bass.MemorySpace.PSUM) as p2,
            tc.tile_pool(name="p3", bufs=2, space=bass.MemorySpace.PSUM) as p3,
            tc.tile_pool(name="pw", bufs=1, space=bass.MemorySpace.PSUM) as pw,
        ):
            w_t = cpool.tile([128, CW], f16)
            nc.sync.dma_start(w_t[:], wts[:])

            W2 = w_t[0:64, _W2_OFF:_W2_OFF + 48]
            W3 = w_t[0:48, _W3_OFF:_W3_OFF + 24]
            B1 = w_t[0:64, _B_OFF + 0:_B_OFF + 2].bitcast(f32)
            B2 = w_t[0:48, _B_OFF + 2:_B_OFF + 4].bitcast(f32)
            B3 = w_t[0:24, _B_OFF + 4:_B_OFF + 6].bitcast(f32)

            def wk(k):
                return w_t[:, k * NW:(k + 1) * NW]

            # PE warm-up: dummy matmuls on the (already loaded) weights so
            # the HAM clock gate reaches 8/8 while the first x chunks DMA.
            wm = pw.tile([NW, BS], f32)
            for _ in range(N_WARMUP):
                nc.tensor.matmul(wm[:], wk(0), w_t[:, 0:BS], start=True, stop=True)

            xt_v = xt.ap().rearrange("(k p) n -> p k n", p=128)  # [128, KT, ROWS]

            rs: dict = {}
            t2s: dict = {}

            def tower2(i):
                # mm2 + tanh for block i (relu(i) finished a block ago, so
                # the PE never stalls on the activation chain)
                z2 = p2.tile([48, BS], f32)
                nc.tensor.matmul(z2[:], W2, rs[i][:], start=True, stop=True)
                t2 = apool.tile([48, BS], f16, tag="t2")
                nc.scalar.activation(t2[:], z2[:], AFT.Tanh, bias=B2)
                t2s[i] = t2

            def tower3(i):
                z3 = p3.tile([24, BS], f32)
                nc.tensor.matmul(z3[:], W3, t2s[i][:], start=True, stop=True)
                t3 = apool.tile([24, BS], f32, tag="t3")
                nc.scalar.activation(t3[:], z3[:], AFT.Tanh, bias=B3)
                nc.scalar.dma_start(out3[:, i * BS:(i + 1) * BS], t3[:])

            for b in range(NBLK):
                cols = slice(b * BS, (b + 1) * BS)
                # stream the block in CH-k-tile chunks so the PE starts as
                # soon as the first chunk lands and DMA never stalls
                chunks = []
                for j in range(NCH):
                    xc = xpool.tile([128, CH, BS], f16, tag="xc")
                    nc.sync.dma_start(
                        xc[:], xt_v[:, j * CH:(j + 1) * CH, cols])
                    chunks.append(xc)

                zt = pz.tile([NW, BS], f32)
                for k in range(KT):
                    nc.tensor.matmul(
                        zt[:], wk(k), chunks[k // CH][:, k % CH, :],
                        start=(k == 0), stop=(k == KT - 1),
                    )

                r = apool.tile([64, BS], f16, tag="r")
                nc.vector.tensor_scalar(
                    r[:], zt[:], B1, 0.0,
                    mybir.AluOpType.add, mybir.AluOpType.max,
                )
                rs[b] = r

                if b >= 1:
                    tower2(b - 1)
                if b >= 2:
                    tower3(b - 2)

            tower2(NBLK - 1)
            tower3(NBLK - 2)
            tower3(NBLK - 1)

    nc.compile()
    return nc


def _get_nc():
    if "nc" not in _STATE:
        _STATE["nc"] = _build_bass()
    return _STATE["nc"]


def _prep(inputs):
    """Host-side folding of the tiny weights + the fp32 u-sgemm."""
    f32 = np.float32
    x = np.asarray(inputs["x"], f32)
    cw = np.asarray(inputs["cross_w"], f32)
    cb = np.asarray(inputs["cross_b"], f32)
    w1 = np.asarray(inputs["w1"], f32)
    b1 = np.asarray(inputs["b1"], f32)
    w2 = np.asarray(inputs["w2"], f32)
    b2 = np.asarray(inputs["b2"], f32)
    w3 = np.asarray(inputs["w3"], f32)
    b3 = np.asarray(inputs["b3"], f32)
    w_out = np.asarray(inputs["w_out"], f32)
    b_out = np.asarray(inputs["b_out"], f32)

    def bn_fold(g, be, m, v):
        a = (np.asarray(g, np.float64) / np.sqrt(np.asarray(v, np.float64) + EPS))
        c = np.asarray(be, np.float64) - a * np.asarray(m, np.float64)
        return a, c

    a1, c1 = bn_fold(inputs["gamma1"], inputs["beta1"], inputs["mean1"], inputs["var1"])
    a2, c2 = bn_fold(inputs["gamma2"], inputs["beta2"], inputs["mean2"], inputs["var2"])
    a3, c3 = bn_fold(inputs["gamma3"], inputs["beta3"], inputs["mean3"], inputs["var3"])

    w_out_x = w_out[:D, 0]
    w_out_h = w_out[D:, 0]

    W2p = (a1[:, None] * w2).astype(f32)                  # [64, 48]
    b2p = (c1 @ w2 + b2).astype(f32)                      # [48]
    W3p = (a2[:, None] * w3).astype(f32)                  # [48, 24]
    b3p = (c2 @ w3 + b3).astype(f32)                      # [24]
    wh = (a3 * w_out_h).astype(f32)                       # [24]
    ch = float(c3 @ w_out_h)

    c01 = float(cb[0] @ cw[1])
    c02 = float(cb[0] @ cw[2])
    c12 = float(cb[1] @ cw[2])
    c3s = float(cb.sum(axis=0) @ w_out_x)

    # the 4 cross dot products, exact fp32 on host (6% of total flops)
    Wc = np.stack([cw[0], cw[1], cw[2], w_out_x], axis=1).astype(f32)   # [D, 4]
    U = x @ Wc                                                          # [B, 4]

    # fused device-side const tensor (fp16)
    wts = np.zeros((128, CW), np.float16)
    wts[:, :KT * NW] = w1.astype(np.float16).reshape(
        KT, 128, NW).transpose(1, 0, 2).reshape(128, -1)
    wts[0:64, _W2_OFF:_W2_OFF + 48] = W2p.astype(np.float16)
    wts[0:48, _W3_OFF:_W3_OFF + 24] = W3p.astype(np.float16)
    wts32 = wts.view(np.float32)
    wts32[0:64, (_B_OFF + 0) // 2] = b1
    wts32[0:48, (_B_OFF + 2) // 2] = b2p
    wts32[0:24, (_B_OFF + 4) // 2] = b3p

    consts = dict(c01=c01, c02=c02, c12=c12, c3s=c3s, ch=ch,
                  b_out=float(b_out[0]), wh=wh, U=U)
    return x, wts, consts


def _combine(t3_all, consts):
    """t3_all: [24, B] device tower output -> final sigmoid output [B, 1]."""
    U = consts["U"].astype(np.float64)
    u0, u1, u2, u3 = U[:, 0], U[:, 1], U[:, 2], U[:, 3]
    hd = consts["wh"].astype(np.float64) @ t3_all.astype(np.float64)     # [B]
    oneS = ((1.0 + u0) * (1.0 + u1) + consts["c01"]) * (1.0 + u2) \
        + consts["c02"] + consts["c12"]
    lin = oneS * u3 + consts["c3s"] + hd + consts["ch"] + consts["b_out"]
    y = 1.0 / (1.0 + np.exp(-lin))
    return y.reshape(-1, 1).astype(np.float32)


def _run(inputs, trace=False, **spmd_kwargs):
    from concourse.bass_utils import run_bass_kernel_spmd

    x, wts, consts = _prep(inputs)
    nc = _get_nc()

    in_maps = []
    for c in range(N_CORES):
        xt_c = np.ascontiguousarray(x[c * ROWS:(c + 1) * ROWS, :].T).astype(np.float16)
        in_maps.append({"xt": xt_c, "wts": wts})

    res = run_bass_kernel_spmd(
        nc, in_maps, core_ids=list(range(N_CORES)), trace=trace, **spmd_kwargs
    )
    t3_all = np.concatenate([r["out3"] for r in res.results], axis=1)  # [24, B]
    return _combine(t3_all, consts), res


def kernel(**inputs) -> np.ndarray:
    y, _ = _run(inputs, trace=False)
    return y
